# revision 30
# baseline (speedup 1.0000x reference)
"""Trainium2 Bass kernel for nn_Net_74552042324489.

Data-parallel over batch n=8 across 8 NeuronCores (1 sample/core).
v2: bf16 inputs (except _4 -> cam, which is tie-sensitive in the
fg-suppression), single weight blob, big prioritized slab DMAs,
f32r cam matmuls, engine-split PSUM copies, N=1024 attention matmuls.

Per-core pipeline:
  cam = fc8_w @ _4 (f32r)      -> norm/suppress -> camT5 = [bg|fg|ones]^T
  x2r = bilinear(x2,112->56)    (bf16 DVE+GpSimd split, align_corners)
  f8_3 = relu(f83_w @ x2r)      f8_4 = relu(f84_w @ deep3)
  x_s = bilinear(x,448->56)     (dense resize-matrix matmuls on PE)
  f = [x_s; f8_3; f8_4] channel-permuted as [f8_4 | f8_3; x_s]
  q,k = Wqk @ f (bf16)
  Attention: S = q^T k blocked [h=128p, k free<=1024]; exp on ScalarE
  (no max-sub: |S|<~30); 2nd matmul lhsT=[camT|ones] fuses numerator +
  softmax denominator; divide at the end.  Output [4, 3136] per core.
"""

import os
import sys

sys.path.insert(0, "/opt/trn_rl_repo")

from contextlib import ExitStack

import numpy as np

import concourse.bass as bass
import concourse.tile as tile
from concourse import bacc, mybir
from concourse.bass_utils import run_bass_kernel_spmd
from concourse.masks import make_identity

F32 = mybir.dt.float32
BF16 = mybir.dt.bfloat16
F32R = mybir.dt.float32r
AF = mybir.ActivationFunctionType
ALU = mybir.AluOpType

HW = 3136  # 56*56
N_CORES = 8

_CACHE = {}

# weight blob column offsets (bf16 blob [128, 1792])
_WB_A112 = 0
_WB_B112 = 56
_WB_F83 = 112
_WB_F84 = 176      # 3 slabs of 128 cols: 176, 304, 432
_WB_QKA = 560
_WB_QKB = 944
_WB_RH = 1328      # [112, 4, 56]
_WB_RW = 1552
_WB_FC8 = 1776     # [128, 4, 8]: per k-slab, cols [hi(4) | lo(4)]
_WB_COLS = 1808


def _resize_mat(h_in: int, h_out: int) -> np.ndarray:
    """Dense [h_in, h_out] bilinear align_corners=True resize matrix."""
    ys = np.linspace(0.0, h_in - 1.0, h_out).astype(np.float32)
    y0 = np.floor(ys).astype(np.int64)
    y1 = np.minimum(y0 + 1, h_in - 1)
    w = (ys - y0).astype(np.float32)
    R = np.zeros((h_in, h_out), np.float32)
    for i in range(h_out):
        R[y0[i], i] += 1.0 - w[i]
        R[y1[i], i] += w[i]
    return R


def _resize_coeffs_112() -> tuple[np.ndarray, np.ndarray]:
    """Per-output-col (0..54) weights for the stride-2 112->56 resize."""
    ys = np.linspace(0.0, 111.0, 56).astype(np.float32)
    y0 = np.floor(ys).astype(np.int64)
    w = (ys - y0).astype(np.float32)
    # structural property (verified): y0[i] == 2i for i < 55; y0[55] == 111
    a = (1.0 - w).astype(np.float32)
    b = w.astype(np.float32)
    return a, b


def _build_program():
    nc = bacc.Bacc(
        "TRN2", target_bir_lowering=False, debug=False, num_devices=N_CORES
    )

    # ---- DRAM I/O ----
    d_wb = nc.dram_tensor("wb16", [128, _WB_COLS], BF16, kind="ExternalInput")
    d_x2 = nc.dram_tensor("x2", [128, 112 * 112], BF16, kind="ExternalInput")
    d_x = nc.dram_tensor("x", [112, 4 * 3 * 448], BF16, kind="ExternalInput")
    d_d3 = nc.dram_tensor("deep3", [320, HW], BF16, kind="ExternalInput")
    d_x4h = nc.dram_tensor("x4h", [512, HW], BF16, kind="ExternalInput")
    d_x4l = nc.dram_tensor("x4l", [512, HW], BF16, kind="ExternalInput")
    d_out = nc.dram_tensor("out", [4, HW], F32, kind="ExternalOutput")

    EPS = 1e-05
    # free-dim chunks of 3136: 3 x 1024 + 64
    NCH = [(0, 1024), (1024, 1024), (2048, 1024), (3072, 64)]
    # h-block partition sizes: 24 x 128 + 1 x 64
    HBLK = [(i * 128, 128) for i in range(24)] + [(3072, 64)]

    def halves(nl):
        # matmul PSUM writes are limited to one 2KB bank (512 fp32 cols)
        return [(0, min(512, nl))] + ([(512, nl - 512)] if nl > 512 else [])

    with tile.TileContext(nc) as tc, ExitStack() as top:
        wpool = top.enter_context(tc.tile_pool(name="wpool", bufs=1))
        persist = top.enter_context(tc.tile_pool(name="persist", bufs=1))
        small = top.enter_context(tc.tile_pool(name="small", bufs=2))

        # ---- persistent SBUF ----
        wsb = wpool.tile([128, _WB_COLS], BF16, tag="wsb")
        ident = wpool.tile([128, 128], F32, tag="ident")
        idb = wpool.tile([128, 128], BF16, tag="idb")

        x2r = persist.tile([128, HW], BF16, tag="x2r")
        f_a = persist.tile([128, HW], BF16, tag="f_a")   # = f8_4
        f_b = persist.tile([67, HW], BF16, tag="f_b")    # = [f8_3(64); x_s(3)]
        qA = persist.tile([128, HW], BF16, tag="qA")
        qB = persist.tile([64, HW], BF16, tag="qB")
        kA = persist.tile([128, HW], BF16, tag="kA")
        kB = persist.tile([64, HW], BF16, tag="kB")
        camT5 = persist.tile([128, 125], BF16, tag="camT5")  # 25 blocks x 5

        a112 = wsb[:, _WB_A112:_WB_A112 + 56]
        b112 = wsb[:, _WB_B112:_WB_B112 + 56]
        f83T = wsb[:, _WB_F83:_WB_F83 + 64]
        f84T = [wsb[:, _WB_F84 + 128 * i:_WB_F84 + 128 * (i + 1)]
                for i in range(3)]
        qkA = wsb[:, _WB_QKA:_WB_QKA + 384]
        qkB = wsb[0:67, _WB_QKB:_WB_QKB + 384]
        rh = wsb[0:112, _WB_RH:_WB_RH + 224].rearrange("p (k o) -> p k o", k=4)
        rw = wsb[0:112, _WB_RW:_WB_RW + 224].rearrange("p (k o) -> p k o", k=4)
        fc8v = wsb[:, _WB_FC8:_WB_FC8 + 32].rearrange("p (k o) -> p k o", k=4)

        x4pool = top.enter_context(tc.tile_pool(name="x4pool", bufs=1))
        x4s = [x4pool.tile([128, HW], BF16, tag=f"x4s{i}", name=f"x4s{i}")
               for i in range(8)]
        camAB = x4pool.tile([8, HW], F32, tag="camAB")
        camBs = x4pool.tile([4, HW], F32, tag="camBs")
        cam = camBs[:]

        with tc.tile_pool(name="p_in", bufs=1) as p_in:
            x2sb = p_in.tile([128, 112 * 112], BF16, tag="x2sb")
            xsb = p_in.tile([112, 4, 3, 448], BF16, tag="xsb")
            d3s = [p_in.tile([128 if i < 2 else 64, HW], BF16, tag=f"d3s{i}",
                             name=f"d3s{i}")
                   for i in range(3)]

            # ---- input DMAs, priority order, all on sync ----
            nc.sync.dma_start(wsb[:], d_wb.ap())
            x2v = x2sb[:].rearrange("p (h w) -> p h w", h=112)
            for ci in range(4):
                nc.sync.dma_start(
                    x2sb[:, ci * 3136:(ci + 1) * 3136],
                    d_x2.ap()[:, ci * 3136:(ci + 1) * 3136],
                )
            nc.sync.dma_start(
                xsb[:], d_x.ap().rearrange("p (k c w) -> p k c w", k=4, c=3)
            )
            for i, (ro, rl) in enumerate(((0, 128), (128, 128), (256, 64))):
                nc.sync.dma_start(d3s[i][0:rl, :], d_d3.ap()[ro:ro + rl, :])
            for i in range(4):
                nc.sync.dma_start(
                    x4s[i][:], d_x4h.ap()[128 * i:128 * (i + 1), :]
                )
            for i in range(4):
                nc.sync.dma_start(
                    x4s[4 + i][:], d_x4l.ap()[128 * i:128 * (i + 1), :]
                )

            make_identity(nc, ident[:])
            nc.vector.tensor_copy(idb[:], ident[:])

            # ======== x2 -> x2r (stride-2 bilinear), DVE/GpSimd split ========
            # W-stage: chunks of 28 h-rows; H-stage consumes matching chunk.
            x2rv = x2r[:].rearrange("p (h w) -> p h w", h=56)
            engs = [nc.vector, nc.gpsimd, nc.vector, nc.gpsimd]
            with tc.tile_pool(name="rzv", bufs=2) as rzv, \
                 tc.tile_pool(name="rzg", bufs=2) as rzg, \
                 tc.tile_pool(name="rzt", bufs=1) as rzt:
                for ci in range(4):
                    eng = engs[ci]
                    pool = rzv if ci % 2 == 0 else rzg
                    sv = x2v[:, 28 * ci:28 * (ci + 1), :]
                    xw = pool.tile([128, 28, 56], BF16, tag="x2w")
                    even = sv[:, :, 0:110:2]
                    odd = sv[:, :, 1:111:2]
                    abc = a112[:, 0:55].unsqueeze(1).broadcast_to([128, 28, 55])
                    bbc = b112[:, 0:55].unsqueeze(1).broadcast_to([128, 28, 55])
                    t1f = rzt.tile([128, 1540], BF16, tag=f"t1{ci % 2}",
                                   name=f"t1{ci % 2}")
                    t2f = rzt.tile([128, 1540], BF16, tag=f"t2{ci % 2}",
                                   name=f"t2{ci % 2}")
                    t1 = t1f[:].rearrange("p (a b) -> p a b", a=28)
                    t2 = t2f[:].rearrange("p (a b) -> p a b", a=28)
                    eng.tensor_tensor(t1, even, abc, op=ALU.mult)
                    eng.tensor_tensor(t2, odd, bbc, op=ALU.mult)
                    eng.tensor_tensor(xw[:, :, 0:55], t1, t2, op=ALU.add)
                    eng.tensor_copy(xw[:, :, 55:56], sv[:, :, 111:112])

                    # H-stage: out rows 14ci .. (13 rows for ci=3, +special 55)
                    jl = 14 if ci < 3 else 13
                    jc = 14 * ci
                    everow = xw[:, 0:2 * jl - 1:2, :]
                    oddrow = xw[:, 1:2 * jl:2, :]
                    arow = a112[:, jc:jc + jl].unsqueeze(2).broadcast_to(
                        [128, jl, 56])
                    brow = b112[:, jc:jc + jl].unsqueeze(2).broadcast_to(
                        [128, jl, 56])
                    t3 = t1f[:, 0:784].rearrange("p (a b) -> p a b", a=14)
                    t4 = t2f[:, 0:784].rearrange("p (a b) -> p a b", a=14)
                    eng.tensor_tensor(t3[:, 0:jl, :], everow, arow, op=ALU.mult)
                    eng.tensor_tensor(t4[:, 0:jl, :], oddrow, brow, op=ALU.mult)
                    eng.tensor_tensor(
                        x2rv[:, jc:jc + jl, :], t3[:, 0:jl, :], t4[:, 0:jl, :],
                        op=ALU.add,
                    )
                    if ci == 3:
                        eng.tensor_copy(x2rv[:, 55:56, :], xw[:, 27:28, :])

            # ======== x -> x_s -> f_b[64:67] (PE resize matmuls) ========
            with tc.tile_pool(name="p4s", bufs=2) as p4s, \
                 tc.tile_pool(name="p4sb", bufs=1) as p4sb, \
                 tc.tile_pool(name="p4p", bufs=1,
                              space=bass.MemorySpace.PSUM) as p4p:
                xh = p4sb.tile([56, 3, 448], BF16, tag="xh")
                xps = [
                    p4p.tile([56, 448], F32, tag=f"xhp{c}", name=f"xhp{c}")
                    for c in range(3)
                ]
                for hc in range(4):
                    for c in range(3):
                        nc.tensor.matmul(
                            xps[c][:], rh[:, hc, :], xsb[:, hc, c, :],
                            start=(hc == 0), stop=(hc == 3),
                        )
                for c in range(3):
                    nc.vector.tensor_copy(xh[:, c, :], xps[c][:])

                xhT = p4sb.tile([112, 12, 56], BF16, tag="xhT")
                for c in range(3):
                    for wc in range(4):
                        tp = p4p.tile([112, 56], BF16, tag="xtp", bufs=2)
                        nc.tensor.transpose(
                            tp[:], xh[:, c, 112 * wc:112 * (wc + 1)],
                            idb[0:56, 0:56],
                        )
                        nc.vector.tensor_copy(xhT[:, c * 4 + wc, :], tp[:])
                xs3 = p4sb.tile([3, HW], BF16, tag="xs3")
                for c in range(3):
                    wp = p4p.tile([56, 56], F32, tag="xwp", bufs=2)
                    for wc in range(4):
                        nc.tensor.matmul(
                            wp[:], xhT[:, c * 4 + wc, :], rw[:, wc, :],
                            start=(wc == 0), stop=(wc == 3),
                        )
                    ws = p4s.tile([56, 56], BF16, tag="xws")
                    nc.vector.tensor_copy(ws[:], wp[:])
                    # gather via DVE-issued DMA: keeps the sync queue free
                    nc.gpsimd.dma_start(xs3[c:c + 1, :], ws[:])
                nc.vector.tensor_copy(f_b[64:67, :], xs3[:])

            # ======== f8_4 = relu(f84T.T @ deep3) -> f_a ========
            with tc.tile_pool(name="p5p", bufs=2,
                              space=bass.MemorySpace.PSUM) as p5p:
                for no, nl in NCH:
                    fp = p5p.tile([128, 1024], F32, tag="f4psum")
                    for ci, cl in ((0, 128), (1, 128), (2, 64)):
                        for h0, hl2 in halves(nl):
                            nc.tensor.matmul(
                                fp[:, h0:h0 + hl2], f84T[ci][0:cl, :],
                                d3s[ci][0:cl, no + h0:no + h0 + hl2],
                                start=(ci == 0), stop=(ci == 2),
                            )
                    nc.scalar.activation(f_a[:, no:no + nl], fp[:, 0:nl],
                                         AF.Relu)

            # ======== f8_3 = relu(f83T.T @ x2r) -> f_b[0:64] ========
            with tc.tile_pool(name="p3p", bufs=2,
                              space=bass.MemorySpace.PSUM) as p3p:
                for no, nl in NCH:
                    fp3 = p3p.tile([64, 1024], F32, tag="f3psum")
                    for h0, hl2 in halves(nl):
                        nc.tensor.matmul(
                            fp3[:, h0:h0 + hl2], f83T[:],
                            x2r[:, no + h0:no + h0 + hl2],
                            start=True, stop=True,
                        )
                    nc.scalar.activation(f_b[0:64, no:no + nl], fp3[:, 0:nl],
                                         AF.Relu)

            # ======== q, k (copies split across ACT/DVE/GpSimd) ========
            with tc.tile_pool(name="p6p", bufs=3,
                              space=bass.MemorySpace.PSUM) as p6p:
                MCH = [(qA, 0, 128, "act"), (qB, 128, 64, "act"),
                       (kA, 192, 128, "dve"), (kB, 320, 64, "dve")]
                for dst, mo, ml, ceng in MCH:
                    for no, nl in NCH:
                        qp = p6p.tile([128, 1024], F32, tag="qkpsum")
                        for h0, hl2 in halves(nl):
                            nc.tensor.matmul(
                                qp[0:ml, h0:h0 + hl2], qkA[:, mo:mo + ml],
                                f_a[:, no + h0:no + h0 + hl2],
                                start=True, stop=False,
                            )
                            nc.tensor.matmul(
                                qp[0:ml, h0:h0 + hl2], qkB[:, mo:mo + ml],
                                f_b[:, no + h0:no + h0 + hl2],
                                start=False, stop=True,
                            )
                        if ceng == "act":
                            nc.scalar.copy(dst[0:ml, no:no + nl],
                                           qp[0:ml, 0:nl])
                        elif ceng == "dve":
                            nc.vector.tensor_copy(dst[0:ml, no:no + nl],
                                                  qp[0:ml, 0:nl])
                        else:
                            nc.gpsimd.tensor_copy(dst[0:ml, no:no + nl],
                                                  qp[0:ml, 0:nl])

            # ======== cam = fc8T.T @ _4 (bf16 hi/lo split, 4-term) ========
            # lhsT [128, 8] = [fc8hi(4) | fc8lo(4)]; two rhs (x4hi, x4lo)
            # per k-slab -> psum rows 0-3 (hi-w) + rows 4-7 (lo-w); DVE adds.
            with tc.tile_pool(name="p1p", bufs=2,
                              space=bass.MemorySpace.PSUM) as p1p:
                for no, nl in NCH:
                    cp = p1p.tile([8, 1024], F32, tag="campsum")
                    for ck in range(4):
                        for h0, hl2 in halves(nl):
                            nc.tensor.matmul(
                                cp[:, h0:h0 + hl2], fc8v[:, ck, :],
                                x4s[ck][:, no + h0:no + h0 + hl2],
                                start=(ck == 0), stop=False,
                            )
                            nc.tensor.matmul(
                                cp[:, h0:h0 + hl2], fc8v[:, ck, :],
                                x4s[4 + ck][:, no + h0:no + h0 + hl2],
                                start=False, stop=(ck == 3),
                            )
                    nc.vector.tensor_copy(camAB[:, no:no + nl], cp[:, 0:nl])
                # partition shift 4..8 -> 0..4 needs a DMA, engines can't
                nc.gpsimd.dma_start(camBs[:], camAB[4:8, :])
                nc.vector.tensor_tensor(
                    camBs[:], camAB[0:4, :], camBs[:], op=ALU.add
                )

        # ======== normalize, transpose, fg-suppress -> camT5 ========
        with tc.tile_pool(name="p1sb", bufs=1) as p1sb, \
             tc.tile_pool(name="p1tp", bufs=2,
                          space=bass.MemorySpace.PSUM) as p1tp:
            mn = small.tile([4, 1], F32, tag="mn")
            mx = small.tile([4, 1], F32, tag="mx")
            nc.vector.tensor_reduce(
                mn[:], cam, axis=mybir.AxisListType.X, op=ALU.min
            )
            nc.vector.tensor_reduce(
                mx[:], cam, axis=mybir.AxisListType.X, op=ALU.max
            )
            rng = small.tile([4, 1], F32, tag="rng")
            nc.vector.tensor_tensor(rng[:], mx[:], mn[:], op=ALU.subtract)
            nc.vector.tensor_scalar_add(rng[:], rng[:], EPS)
            rs = small.tile([4, 1], F32, tag="rs")
            nc.vector.reciprocal(rs[:], rng[:])
            norm = p1sb.tile([4, HW], F32, tag="norm")
            nc.vector.tensor_scalar(
                norm[:], cam, mn[:], rs[:], op0=ALU.subtract, op1=ALU.mult
            )

            camTall = p1sb.tile([128, 25, 4], F32, tag="camTall")
            nc.vector.memset(camTall[64:128, 24, :], 0.0)
            for bi, (ho, hl) in enumerate(HBLK):
                tp = p1tp.tile([128, 4], F32, tag="tpsum")
                nc.tensor.transpose(
                    tp[0:hl, :], norm[:, ho:ho + hl], ident[0:4, 0:4]
                )
                nc.vector.tensor_copy(camTall[0:hl, bi, :], tp[0:hl, :])
            c5v = camT5[:].rearrange("p (b f) -> p b f", f=5)
            nc.vector.memset(c5v[:, :, 4], 1.0)
            fm = p1sb.tile([128, 25], F32, tag="fm")
            nc.vector.tensor_reduce(
                fm[:], camTall[:, :, 1:4], axis=mybir.AxisListType.X, op=ALU.max
            )
            nc.vector.tensor_scalar(
                c5v[:, :, 0], fm[:], -1.0, 1.0, op0=ALU.mult, op1=ALU.add
            )
            msk = p1sb.tile([128, 25, 3], F32, tag="msk")
            fmb = fm[:].unsqueeze(2).broadcast_to([128, 25, 3])
            nc.vector.tensor_tensor(
                msk[:], camTall[:, :, 1:4], fmb, op=ALU.is_ge
            )
            nc.vector.tensor_tensor(
                c5v[:, :, 1:4], camTall[:, :, 1:4], msk[:], op=ALU.mult
            )

        # ======== attention ========
        with tc.tile_pool(name="p7e", bufs=6) as p7e, \
             tc.tile_pool(name="p7r", bufs=2) as p7r, \
             tc.tile_pool(name="p7sb", bufs=1) as p7sb, \
             tc.tile_pool(name="p7s", bufs=3, space=bass.MemorySpace.PSUM) as p7s, \
             tc.tile_pool(name="p7o", bufs=1, space=bass.MemorySpace.PSUM) as p7o:
            out_sb = p7sb.tile([4, HW], F32, tag="out_sb")
            for ko, kl in NCH:
                pout = p7o.tile([5, 1024], F32, tag="pout")
                for bi, (ho, hl) in enumerate(HBLK):
                    sp = p7s.tile([128, 1024], F32, tag="spsum")
                    for h0, hl2 in halves(kl):
                        nc.tensor.matmul(
                            sp[0:hl, h0:h0 + hl2], qA[:, ho:ho + hl],
                            kA[:, ko + h0:ko + h0 + hl2],
                            start=True, stop=False,
                        )
                        nc.tensor.matmul(
                            sp[0:hl, h0:h0 + hl2], qB[:, ho:ho + hl],
                            kB[:, ko + h0:ko + h0 + hl2],
                            start=False, stop=True,
                        )
                    et = p7e.tile([128, 1024], BF16, tag="exptile")
                    nc.scalar.activation(et[0:hl, 0:kl], sp[0:hl, 0:kl], AF.Exp)
                    nkb = (kl + 511) // 512
                    for kb in range(nkb):
                        kbl = min(512, kl - kb * 512)
                        nc.tensor.matmul(
                            pout[:, kb * 512:kb * 512 + kbl],
                            camT5[0:hl, bi * 5:bi * 5 + 5],
                            et[0:hl, kb * 512:kb * 512 + kbl],
                            start=(bi == 0), stop=(bi == 24),
                        )
                ot5 = p7r.tile([5, 1024], F32, tag="ot5")
                nc.vector.tensor_copy(ot5[:, 0:kl], pout[:, 0:kl])
                den = p7r.tile([1, 1024], F32, tag="den")
                nc.gpsimd.dma_start(den[0:1, 0:kl], ot5[4:5, 0:kl])
                rcp = p7r.tile([1, 1024], F32, tag="rcp")
                rsc = p7r.tile([1, 1024], F32, tag="rsc")
                nc.vector.reciprocal_approx_accurate(
                    rcp[0:1, 0:kl], den[0:1, 0:kl], rsc[0:1, 0:kl]
                )
                rb = p7r.tile([4, 1024], F32, tag="rb")
                nc.gpsimd.partition_broadcast(rb[:, 0:kl], rcp[0:1, 0:kl])
                nc.gpsimd.tensor_tensor(
                    out_sb[:, ko:ko + kl], ot5[0:4, 0:kl], rb[:, 0:kl],
                    op=ALU.mult,
                )
                nc.gpsimd.dma_start(
                    d_out.ap()[:, ko:ko + kl], out_sb[:, ko:ko + kl]
                )

    nc.compile()
    return nc


def _get_program():
    if "nc" not in _CACHE:
        _CACHE["nc"] = _build_program()
    return _CACHE["nc"]


def _host_prep(inputs: dict) -> list[dict]:
    import ml_dtypes

    BFNP = ml_dtypes.bfloat16
    x = np.asarray(inputs["x"], np.float32)
    x2 = np.asarray(inputs["x2"], np.float32)
    deep3 = np.asarray(inputs["deep3"], np.float32)
    _4 = np.ascontiguousarray(np.asarray(inputs["_4"], np.float32))
    fc8_w = np.asarray(inputs["fc8_w"], np.float32)
    f83_w = np.asarray(inputs["f83_w"], np.float32)
    f84_w = np.asarray(inputs["f84_w"], np.float32)
    f91_w = np.asarray(inputs["f91_w"], np.float32)
    f92_w = np.asarray(inputs["f92_w"], np.float32)

    n = x.shape[0]
    # weight blob [128, _WB_COLS] bf16
    wb = np.zeros((128, _WB_COLS), np.float32)
    a112, b112 = _resize_coeffs_112()
    wb[:, _WB_A112:_WB_A112 + 56] = np.broadcast_to(a112, (128, 56))
    wb[:, _WB_B112:_WB_B112 + 56] = np.broadcast_to(b112, (128, 56))
    wb[:, _WB_F83:_WB_F83 + 64] = f83_w.T                     # [128, 64]
    f84T = f84_w.T                                            # [320, 128]
    wb[:, _WB_F84 + 0:_WB_F84 + 128] = f84T[0:128]
    wb[:, _WB_F84 + 128:_WB_F84 + 256] = f84T[128:256]
    wb[0:64, _WB_F84 + 256:_WB_F84 + 384] = f84T[256:320]
    # f channel permutation: [f8_4 (128), f8_3 (64), x_s (3)]
    perm = np.concatenate([np.arange(67, 195), np.arange(3, 67), np.arange(3)])
    wqkT = np.concatenate([f91_w, f92_w], axis=0)[:, perm].T  # [195, 384]
    wb[:, _WB_QKA:_WB_QKA + 384] = wqkT[0:128]
    wb[0:67, _WB_QKB:_WB_QKB + 384] = wqkT[128:195]
    rh = _resize_mat(448, 56)  # [448, 56]; same matrix for H and W
    rhr = rh.reshape(4, 112, 56).transpose(1, 0, 2).reshape(112, 224)
    wb[0:112, _WB_RH:_WB_RH + 224] = rhr
    wb[0:112, _WB_RW:_WB_RW + 224] = rhr
    # fc8 bf16 hi/lo packed [128, 4ck, 8]: cols [hi(4) | lo(4)]
    fhi = fc8_w.astype(BFNP).astype(np.float32)
    flo = fc8_w - fhi
    fhiT = fhi.T.reshape(4, 128, 4).transpose(1, 0, 2)  # [p, ck, o]
    floT = flo.T.reshape(4, 128, 4).transpose(1, 0, 2)
    wb[:, _WB_FC8:_WB_FC8 + 32] = np.concatenate(
        [fhiT, floT], axis=2).reshape(128, 32)
    wb = wb.astype(BFNP)

    xb = np.ascontiguousarray(
        x.reshape(n, 3, 4, 112, 448).transpose(0, 3, 2, 1, 4)
        .reshape(n, 112, 4 * 3 * 448).astype(BFNP))
    x2b = np.ascontiguousarray(x2.reshape(n, 128, 112 * 112).astype(BFNP))
    d3b = np.ascontiguousarray(deep3.reshape(n, 320, HW).astype(BFNP))
    x4f = _4.reshape(n, 512, HW)
    x4h = x4f.astype(BFNP)
    x4l = (x4f - x4h.astype(np.float32)).astype(BFNP)

    shared = {"wb16": wb}
    in_maps = []
    for i in range(n):
        m = dict(shared)
        m["x2"] = x2b[i]
        m["x"] = xb[i]
        m["deep3"] = d3b[i]
        m["x4h"] = np.ascontiguousarray(x4h[i])
        m["x4l"] = np.ascontiguousarray(x4l[i])
        in_maps.append(m)
    return in_maps


def _install_ntff_hook() -> bool:
    """Register the NTFF profile hook that the agent image's antenv lacks."""
    try:
        import types

        import antenv

        if "antenv.axon_hooks" not in sys.modules:
            mod = types.ModuleType("antenv.axon_hooks")
            store = {"h": None}
            mod.set_axon_ntff_profile_hook = lambda h: store.update(h=h)
            mod.get_axon_ntff_profile_hook = lambda: store["h"]
            sys.modules["antenv.axon_hooks"] = mod
            antenv.axon_hooks = mod
            from trn_agent_boot.trn_boot import _ntff_profile_via_ctypes

            hook = _ntff_profile_via_ctypes("/opt/axon/libaxon_pjrt.so")
            if hook is None:
                return False
            mod.set_axon_ntff_profile_hook(hook)
        return sys.modules["antenv.axon_hooks"].get_axon_ntff_profile_hook() is not None
    except Exception as e:  # profiling is best-effort
        print(f"ntff hook install failed: {e}", file=sys.stderr)
        return False


def kernel(**inputs) -> np.ndarray:
    nc = _get_program()
    in_maps = _host_prep(inputs)
    trace = bool(int(os.environ.get("KERNEL_PROFILE", "0")))
    if trace:
        trace = _install_ntff_hook()
    res = run_bass_kernel_spmd(nc, in_maps, core_ids=list(range(N_CORES)),
                               trace=trace)
    _CACHE["last_result"] = res
    out = np.stack([r["out"] for r in res.results]).reshape(8, 4, 56, 56)
    return out.astype(np.float32)


# revision 34
# speedup vs baseline: 1.1192x; 1.1192x over previous
"""Trainium2 Bass kernel for nn_Net_74552042324489.

Data-parallel over batch n=8 across 8 NeuronCores (1 sample/core).
v3: bf16 inputs; x2 bilinear resize as host-GATHERED quadrants (pure
indexing) x constant coeff tensors -> 7 flat DVE/GpSimd ops; cam via
bf16 hi/lo split (tie-sensitive fg-suppression needs ~fp32); cam fold +
min/max overlapped per chunk; normalization applied post-transpose on
the tiny [128,25,4] layout; software-pipelined attention emission so
the PE never idles (idle windows drop the HAM clock to 1.2 GHz).

Per-core pipeline:
  cam = fc8_w @ _4 (bf16 hi/lo 4-term) -> norm/suppress -> camT5
  x2r = bilinear(x2,112->56)  f8_3 = relu(f83_w @ x2r)
  f8_4 = relu(f84_w @ deep3)  x_s = bilinear(x,448->56) (PE matmuls)
  f = [x_s; f8_3; f8_4] channel-permuted as [f8_4 | f8_3; x_s]
  q,k = Wqk @ f (bf16)
  S = q^T k blocked [h<=128p, k<=1024]; exp on ScalarE (|S|<~30);
  2nd matmul lhsT=[camT|ones] fuses numerator + denominator; divide.
"""

import os
import sys

sys.path.insert(0, "/opt/trn_rl_repo")

from contextlib import ExitStack

import numpy as np

import concourse.bass as bass
import concourse.tile as tile
from concourse import bacc, mybir
from concourse.bass_utils import run_bass_kernel_spmd
from concourse.masks import make_identity

F32 = mybir.dt.float32
BF16 = mybir.dt.bfloat16
AF = mybir.ActivationFunctionType
ALU = mybir.AluOpType

HW = 3136  # 56*56
N_CORES = 8

_CACHE = {}

# weight blob column offsets (bf16 blob [128, _WB_COLS])
_WB_F83 = 0
_WB_F84 = 64       # 3 slabs of 128 cols
_WB_QKA = 448
_WB_QKB = 832
_WB_RH = 1216      # [112, 4, 56]
_WB_RW = 1440
_WB_FC8 = 1664     # [128, 4, 8]: per k-slab, cols [hi(4) | lo(4)]
_WB_COLS = 1696


def _resize_grid(h_in: int, h_out: int):
    ys = np.linspace(0.0, h_in - 1.0, h_out)
    y0 = np.floor(ys).astype(np.int64)
    y1 = np.minimum(y0 + 1, h_in - 1)
    w = (ys - y0).astype(np.float32)
    return y0, y1, w


def _resize_mat(h_in: int, h_out: int) -> np.ndarray:
    """Dense [h_in, h_out] bilinear align_corners=True resize matrix."""
    y0, y1, w = _resize_grid(h_in, h_out)
    R = np.zeros((h_in, h_out), np.float32)
    for i in range(h_out):
        R[y0[i], i] += 1.0 - w[i]
        R[y1[i], i] += w[i]
    return R


def _build_program():
    nc = bacc.Bacc(
        "TRN2", target_bir_lowering=False, debug=False, num_devices=N_CORES
    )

    # ---- DRAM I/O ----
    d_wb = nc.dram_tensor("wb16", [128, _WB_COLS], BF16, kind="ExternalInput")
    d_cq = nc.dram_tensor("cq", [128, 4 * HW], BF16, kind="ExternalInput")
    d_x2q = nc.dram_tensor("x2q", [128, 4 * HW], BF16, kind="ExternalInput")
    d_x = nc.dram_tensor("x", [112, 4 * 3 * 448], BF16, kind="ExternalInput")
    d_d3 = nc.dram_tensor("deep3", [320, HW], BF16, kind="ExternalInput")
    d_x4h = nc.dram_tensor("x4h", [512, HW], BF16, kind="ExternalInput")
    d_x4l = nc.dram_tensor("x4l", [512, HW], BF16, kind="ExternalInput")
    d_out = nc.dram_tensor("out", [4, HW], F32, kind="ExternalOutput")

    EPS = 1e-05
    # free-dim chunks of 3136: 3 x 1024 + 64
    NCH = [(0, 1024), (1024, 1024), (2048, 1024), (3072, 64)]
    # h-block partition sizes: 24 x 128 + 1 x 64
    HBLK = [(i * 128, 128) for i in range(24)] + [(3072, 64)]

    def halves(nl):
        # matmul PSUM writes are limited to one 2KB bank (512 fp32 cols)
        return [(0, min(512, nl))] + ([(512, nl - 512)] if nl > 512 else [])

    with tile.TileContext(nc) as tc, ExitStack() as top:
        wpool = top.enter_context(tc.tile_pool(name="wpool", bufs=1))
        persist = top.enter_context(tc.tile_pool(name="persist", bufs=1))
        small = top.enter_context(tc.tile_pool(name="small", bufs=2))

        # ---- persistent SBUF ----
        wsb = wpool.tile([128, _WB_COLS], BF16, tag="wsb")
        ident = wpool.tile([128, 128], F32, tag="ident")
        idb = wpool.tile([128, 128], BF16, tag="idb")

        x2r = persist.tile([128, HW], BF16, tag="x2r")
        f_a = persist.tile([128, HW], BF16, tag="f_a")   # = f8_4
        f_b = persist.tile([67, HW], BF16, tag="f_b")    # = [f8_3(64); x_s(3)]
        qA = persist.tile([128, HW], BF16, tag="qA")
        qB = persist.tile([64, HW], BF16, tag="qB")
        kA = persist.tile([128, HW], BF16, tag="kA")
        kB = persist.tile([64, HW], BF16, tag="kB")
        camT5 = persist.tile([128, 125], BF16, tag="camT5")  # 25 blocks x 5

        f83T = wsb[:, _WB_F83:_WB_F83 + 64]
        f84T = [wsb[:, _WB_F84 + 128 * i:_WB_F84 + 128 * (i + 1)]
                for i in range(3)]
        qkA = wsb[:, _WB_QKA:_WB_QKA + 384]
        qkB = wsb[0:67, _WB_QKB:_WB_QKB + 384]
        rh = wsb[0:112, _WB_RH:_WB_RH + 224].rearrange("p (k o) -> p k o", k=4)
        rw = wsb[0:112, _WB_RW:_WB_RW + 224].rearrange("p (k o) -> p k o", k=4)
        fc8v = wsb[:, _WB_FC8:_WB_FC8 + 32].rearrange("p (k o) -> p k o", k=4)

        x4pool = top.enter_context(tc.tile_pool(name="x4pool", bufs=1))
        x4s = [x4pool.tile([128, HW], BF16, tag=f"x4s{i}", name=f"x4s{i}")
               for i in range(8)]
        camAB = x4pool.tile([8, HW], F32, tag="camAB")
        camBs = x4pool.tile([4, HW], F32, tag="camBs")
        cam = camBs[:]
        pmn = x4pool.tile([4, 4], F32, tag="pmn")
        pmx = x4pool.tile([4, 4], F32, tag="pmx")

        with tc.tile_pool(name="p_in", bufs=1) as p_in:
            xsb = p_in.tile([112, 4, 3, 448], BF16, tag="xsb")
            d3s = [p_in.tile([128 if i < 2 else 64, HW], BF16, tag=f"d3s{i}",
                             name=f"d3s{i}")
                   for i in range(3)]

            with tc.tile_pool(name="cqp", bufs=1) as cqp:
                cqs = cqp.tile([128, 4, HW], BF16, tag="cqs")
                x2qs = cqp.tile([128, 4, HW], BF16, tag="x2qs")

                # ---- input DMAs, priority order, all on sync ----
                nc.sync.dma_start(wsb[:], d_wb.ap())
                nc.sync.dma_start(
                    cqs[:], d_cq.ap().rearrange("p (q f) -> p q f", q=4))
                nc.sync.dma_start(
                    x2qs[:], d_x2q.ap().rearrange("p (q f) -> p q f", q=4))
                nc.sync.dma_start(
                    xsb[:], d_x.ap().rearrange("p (k c w) -> p k c w",
                                               k=4, c=3)
                )
                for i, (ro, rl) in enumerate(((0, 128), (128, 128),
                                              (256, 64))):
                    nc.sync.dma_start(d3s[i][0:rl, :],
                                      d_d3.ap()[ro:ro + rl, :])
                for i in range(4):
                    nc.sync.dma_start(
                        x4s[i][:], d_x4h.ap()[128 * i:128 * (i + 1), :]
                    )
                for i in range(4):
                    nc.sync.dma_start(
                        x4s[4 + i][:], d_x4l.ap()[128 * i:128 * (i + 1), :]
                    )

                make_identity(nc, ident[:])
                nc.vector.tensor_copy(idb[:], ident[:])

                # ======== x2 -> x2r: sum of 4 pre-gathered quadrants =====
                # in-place in the quadrant buffer: no temp SBUF
                q0, q1 = x2qs[:, 0, :], x2qs[:, 1, :]
                q2, q3 = x2qs[:, 2, :], x2qs[:, 3, :]
                nc.vector.tensor_tensor(q0, q0, cqs[:, 0, :], op=ALU.mult)
                nc.gpsimd.tensor_tensor(q1, q1, cqs[:, 1, :], op=ALU.mult)
                nc.vector.tensor_tensor(q2, q2, cqs[:, 2, :], op=ALU.mult)
                nc.gpsimd.tensor_tensor(q3, q3, cqs[:, 3, :], op=ALU.mult)
                nc.vector.tensor_tensor(q0, q0, q1, op=ALU.add)
                nc.gpsimd.tensor_tensor(q3, q2, q3, op=ALU.add)
                nc.vector.tensor_tensor(x2r[:], q0, q3, op=ALU.add)

            # ======== x -> x_s -> f_b[64:67] (PE resize matmuls) ========
            with tc.tile_pool(name="p4s", bufs=2) as p4s, \
                 tc.tile_pool(name="p4sb", bufs=1) as p4sb, \
                 tc.tile_pool(name="p4p", bufs=1,
                              space=bass.MemorySpace.PSUM) as p4p:
                xh = p4sb.tile([56, 3, 448], BF16, tag="xh")
                xps = [
                    p4p.tile([56, 448], F32, tag=f"xhp{c}", name=f"xhp{c}")
                    for c in range(3)
                ]
                for hc in range(4):
                    for c in range(3):
                        nc.tensor.matmul(
                            xps[c][:], rh[:, hc, :], xsb[:, hc, c, :],
                            start=(hc == 0), stop=(hc == 3),
                        )
                for c in range(3):
                    nc.vector.tensor_copy(xh[:, c, :], xps[c][:])

                xhT = p4sb.tile([112, 12, 56], BF16, tag="xhT")
                for c in range(3):
                    for wc in range(4):
                        tp = p4p.tile([112, 56], BF16, tag="xtp", bufs=2)
                        nc.tensor.transpose(
                            tp[:], xh[:, c, 112 * wc:112 * (wc + 1)],
                            idb[0:56, 0:56],
                        )
                        nc.vector.tensor_copy(xhT[:, c * 4 + wc, :], tp[:])
                xs3 = p4sb.tile([3, HW], BF16, tag="xs3")
                for c in range(3):
                    wp = p4p.tile([56, 56], F32, tag="xwp", bufs=2)
                    for wc in range(4):
                        nc.tensor.matmul(
                            wp[:], xhT[:, c * 4 + wc, :], rw[:, wc, :],
                            start=(wc == 0), stop=(wc == 3),
                        )
                    ws = p4s.tile([56, 56], BF16, tag="xws")
                    nc.vector.tensor_copy(ws[:], wp[:])
                    # gather via gpsimd-issued DMA: keeps sync queue free
                    nc.gpsimd.dma_start(xs3[c:c + 1, :], ws[:])
                nc.vector.tensor_copy(f_b[64:67, :], xs3[:])

            # ======== f8_4 = relu(f84T.T @ deep3) -> f_a ========
            with tc.tile_pool(name="p5p", bufs=2,
                              space=bass.MemorySpace.PSUM) as p5p:
                for no, nl in NCH:
                    fp = p5p.tile([128, 1024], F32, tag="f4psum")
                    for ci, cl in ((0, 128), (1, 128), (2, 64)):
                        for h0, hl2 in halves(nl):
                            nc.tensor.matmul(
                                fp[:, h0:h0 + hl2], f84T[ci][0:cl, :],
                                d3s[ci][0:cl, no + h0:no + h0 + hl2],
                                start=(ci == 0), stop=(ci == 2),
                            )
                    nc.scalar.activation(f_a[:, no:no + nl], fp[:, 0:nl],
                                         AF.Relu)

            # ======== f8_3 = relu(f83T.T @ x2r) -> f_b[0:64] ========
            with tc.tile_pool(name="p3p", bufs=2,
                              space=bass.MemorySpace.PSUM) as p3p:
                for no, nl in NCH:
                    fp3 = p3p.tile([64, 1024], F32, tag="f3psum")
                    for h0, hl2 in halves(nl):
                        nc.tensor.matmul(
                            fp3[:, h0:h0 + hl2], f83T[:],
                            x2r[:, no + h0:no + h0 + hl2],
                            start=True, stop=True,
                        )
                    nc.scalar.activation(f_b[0:64, no:no + nl], fp3[:, 0:nl],
                                         AF.Relu)

            # ======== q, k (copies split across ACT/DVE) ========
            with tc.tile_pool(name="p6p", bufs=3,
                              space=bass.MemorySpace.PSUM) as p6p:
                MCH = [(qA, 0, 128, "act"), (qB, 128, 64, "act"),
                       (kA, 192, 128, "dve"), (kB, 320, 64, "dve")]
                for dst, mo, ml, ceng in MCH:
                    for no, nl in NCH:
                        qp = p6p.tile([128, 1024], F32, tag="qkpsum")
                        for h0, hl2 in halves(nl):
                            nc.tensor.matmul(
                                qp[0:ml, h0:h0 + hl2], qkA[:, mo:mo + ml],
                                f_a[:, no + h0:no + h0 + hl2],
                                start=True, stop=False,
                            )
                            nc.tensor.matmul(
                                qp[0:ml, h0:h0 + hl2], qkB[:, mo:mo + ml],
                                f_b[:, no + h0:no + h0 + hl2],
                                start=False, stop=True,
                            )
                        if ceng == "act":
                            nc.scalar.copy(dst[0:ml, no:no + nl],
                                           qp[0:ml, 0:nl])
                        else:
                            nc.vector.tensor_copy(dst[0:ml, no:no + nl],
                                                  qp[0:ml, 0:nl])

            # ======== cam = fc8T.T @ _4 (bf16 hi/lo, 4-term) ========
            # lhsT [128, 8] = [fc8hi(4) | fc8lo(4)]; per-chunk fold +
            # running min/max so the norm chain is off the critical path.
            with tc.tile_pool(name="p1p", bufs=2,
                              space=bass.MemorySpace.PSUM) as p1p:
                for idx, (no, nl) in enumerate(NCH):
                    cp = p1p.tile([8, 1024], F32, tag="campsum")
                    for ck in range(4):
                        for h0, hl2 in halves(nl):
                            nc.tensor.matmul(
                                cp[:, h0:h0 + hl2], fc8v[:, ck, :],
                                x4s[ck][:, no + h0:no + h0 + hl2],
                                start=(ck == 0), stop=False,
                            )
                            nc.tensor.matmul(
                                cp[:, h0:h0 + hl2], fc8v[:, ck, :],
                                x4s[4 + ck][:, no + h0:no + h0 + hl2],
                                start=False, stop=(ck == 3),
                            )
                    nc.vector.tensor_copy(camAB[:, no:no + nl], cp[:, 0:nl])
                    # partition shift 4..8 -> 0..4 needs a DMA
                    nc.gpsimd.dma_start(camBs[:, no:no + nl],
                                        camAB[4:8, no:no + nl])
                    nc.vector.tensor_tensor(
                        camBs[:, no:no + nl], camAB[0:4, no:no + nl],
                        camBs[:, no:no + nl], op=ALU.add,
                    )
                    nc.vector.tensor_reduce(
                        pmn[:, idx:idx + 1], camBs[:, no:no + nl],
                        axis=mybir.AxisListType.X, op=ALU.min,
                    )
                    nc.vector.tensor_reduce(
                        pmx[:, idx:idx + 1], camBs[:, no:no + nl],
                        axis=mybir.AxisListType.X, op=ALU.max,
                    )

        # ======== transpose raw cam; normalize + suppress on [128,25,4] ====
        with tc.tile_pool(name="p1sb", bufs=1) as p1sb, \
             tc.tile_pool(name="p1tp", bufs=2,
                          space=bass.MemorySpace.PSUM) as p1tp:
            mn = small.tile([4, 1], F32, tag="mn")
            mx = small.tile([4, 1], F32, tag="mx")
            nc.vector.tensor_reduce(
                mn[:], pmn[:], axis=mybir.AxisListType.X, op=ALU.min
            )
            nc.vector.tensor_reduce(
                mx[:], pmx[:], axis=mybir.AxisListType.X, op=ALU.max
            )
            rng = small.tile([4, 1], F32, tag="rng")
            nc.vector.tensor_tensor(rng[:], mx[:], mn[:], op=ALU.subtract)
            nc.vector.tensor_scalar_add(rng[:], rng[:], EPS)
            rs = small.tile([4, 1], F32, tag="rs")
            nc.vector.reciprocal(rs[:], rng[:])
            # pack [mn|rs] -> [4,2] -> DMA flatten -> [1,8] -> all partitions
            mr = p1sb.tile([4, 2], F32, tag="mr")
            nc.vector.tensor_copy(mr[:, 0:1], mn[:])
            nc.vector.tensor_copy(mr[:, 1:2], rs[:])
            mrT = p1sb.tile([1, 8], F32, tag="mrT")
            nc.gpsimd.dma_start(mrT[:], mr[:])
            pb = p1sb.tile([128, 8], F32, tag="pb")
            nc.gpsimd.partition_broadcast(pb[:], mrT[:])
            pbv = pb[:].rearrange("p (c two) -> p c two", two=2)
            mnb = pbv[:, :, 0].unsqueeze(1).broadcast_to([128, 25, 4])
            rsb = pbv[:, :, 1].unsqueeze(1).broadcast_to([128, 25, 4])

            camTall = p1sb.tile([128, 25, 4], F32, tag="camTall")
            nc.vector.memset(camTall[64:128, 24, :], 0.0)
            for bi, (ho, hl) in enumerate(HBLK):
                tp = p1tp.tile([128, 4], F32, tag="tpsum")
                nc.tensor.transpose(
                    tp[0:hl, :], cam[:, ho:ho + hl], ident[0:4, 0:4]
                )
                nc.scalar.copy(camTall[0:hl, bi, :], tp[0:hl, :])
            normT = p1sb.tile([128, 25, 4], F32, tag="normT")
            nc.vector.tensor_tensor(normT[:], camTall[:], mnb, op=ALU.subtract)
            nc.vector.tensor_tensor(normT[:], normT[:], rsb, op=ALU.mult)

            c5v = camT5[:].rearrange("p (b f) -> p b f", f=5)
            nc.vector.memset(c5v[:, :, 4], 1.0)
            fm = p1sb.tile([128, 25], F32, tag="fm")
            nc.vector.tensor_reduce(
                fm[:], normT[:, :, 1:4], axis=mybir.AxisListType.X, op=ALU.max
            )
            nc.vector.tensor_scalar(
                c5v[:, :, 0], fm[:], -1.0, 1.0, op0=ALU.mult, op1=ALU.add
            )
            msk = p1sb.tile([128, 25, 3], F32, tag="msk")
            fmb = fm[:].unsqueeze(2).broadcast_to([128, 25, 3])
            nc.vector.tensor_tensor(
                msk[:], normT[:, :, 1:4], fmb, op=ALU.is_ge
            )
            nc.vector.tensor_tensor(
                c5v[:, :, 1:4], normT[:, :, 1:4], msk[:], op=ALU.mult
            )

        # ======== attention (software-pipelined emission) ========
        with tc.tile_pool(name="p7e", bufs=10) as p7e, \
             tc.tile_pool(name="p7r", bufs=2) as p7r, \
             tc.tile_pool(name="p7sb", bufs=1) as p7sb, \
             tc.tile_pool(name="p7s", bufs=3, space=bass.MemorySpace.PSUM) as p7s, \
             tc.tile_pool(name="p7o", bufs=1, space=bass.MemorySpace.PSUM) as p7o:
            out_sb = p7sb.tile([4, HW], F32, tag="out_sb")
            for ki, (ko, kl) in enumerate(NCH):
                pout = p7o.tile([5, 1024], F32, tag="pout")
                depth = 8 if ki == 0 else 2
                ets = {}

                def emit_num(bj, ko=ko, kl=kl, ets=ets, pout=pout):
                    et_, hl_ = ets.pop(bj)
                    for kb0, kbl in halves(kl):
                        nc.tensor.matmul(
                            pout[:, kb0:kb0 + kbl],
                            camT5[0:hl_, bj * 5:bj * 5 + 5],
                            et_[0:hl_, kb0:kb0 + kbl],
                            start=(bj == 0), stop=(bj == 24),
                        )

                for bi, (ho, hl) in enumerate(HBLK):
                    sp = p7s.tile([128, 1024], F32, tag="spsum")
                    for h0, hl2 in halves(kl):
                        nc.tensor.matmul(
                            sp[0:hl, h0:h0 + hl2], qA[:, ho:ho + hl],
                            kA[:, ko + h0:ko + h0 + hl2],
                            start=True, stop=False,
                        )
                        nc.tensor.matmul(
                            sp[0:hl, h0:h0 + hl2], qB[:, ho:ho + hl],
                            kB[:, ko + h0:ko + h0 + hl2],
                            start=False, stop=True,
                        )
                    et = p7e.tile([128, 1024], BF16, tag="exptile")
                    nc.scalar.activation(et[0:hl, 0:kl], sp[0:hl, 0:kl], AF.Exp)
                    ets[bi] = (et, hl)
                    if bi >= depth:
                        emit_num(bi - depth)
                for bj in range(25 - depth, 25):
                    emit_num(bj)

                ot5 = p7r.tile([5, 1024], F32, tag="ot5")
                nc.vector.tensor_copy(ot5[:, 0:kl], pout[:, 0:kl])
                den = p7r.tile([1, 1024], F32, tag="den")
                nc.gpsimd.dma_start(den[0:1, 0:kl], ot5[4:5, 0:kl])
                rcp = p7r.tile([1, 1024], F32, tag="rcp")
                rsc = p7r.tile([1, 1024], F32, tag="rsc")
                nc.vector.reciprocal_approx_accurate(
                    rcp[0:1, 0:kl], den[0:1, 0:kl], rsc[0:1, 0:kl]
                )
                rb = p7r.tile([4, 1024], F32, tag="rb")
                nc.gpsimd.partition_broadcast(rb[:, 0:kl], rcp[0:1, 0:kl])
                nc.gpsimd.tensor_tensor(
                    out_sb[:, ko:ko + kl], ot5[0:4, 0:kl], rb[:, 0:kl],
                    op=ALU.mult,
                )
                nc.gpsimd.dma_start(
                    d_out.ap()[:, ko:ko + kl], out_sb[:, ko:ko + kl]
                )

    nc.compile()
    return nc


def _get_program():
    if "nc" not in _CACHE:
        _CACHE["nc"] = _build_program()
    return _CACHE["nc"]


def _host_prep(inputs: dict) -> list[dict]:
    import ml_dtypes

    BFNP = ml_dtypes.bfloat16
    x = np.asarray(inputs["x"], np.float32)
    x2 = np.asarray(inputs["x2"], np.float32)
    deep3 = np.asarray(inputs["deep3"], np.float32)
    _4 = np.ascontiguousarray(np.asarray(inputs["_4"], np.float32))
    fc8_w = np.asarray(inputs["fc8_w"], np.float32)
    f83_w = np.asarray(inputs["f83_w"], np.float32)
    f84_w = np.asarray(inputs["f84_w"], np.float32)
    f91_w = np.asarray(inputs["f91_w"], np.float32)
    f92_w = np.asarray(inputs["f92_w"], np.float32)

    n = x.shape[0]
    # weight blob [128, _WB_COLS] bf16
    wb = np.zeros((128, _WB_COLS), np.float32)
    wb[:, _WB_F83:_WB_F83 + 64] = f83_w.T                     # [128, 64]
    f84T = f84_w.T                                            # [320, 128]
    wb[:, _WB_F84 + 0:_WB_F84 + 128] = f84T[0:128]
    wb[:, _WB_F84 + 128:_WB_F84 + 256] = f84T[128:256]
    wb[0:64, _WB_F84 + 256:_WB_F84 + 384] = f84T[256:320]
    # f channel permutation: [f8_4 (128), f8_3 (64), x_s (3)]
    perm = np.concatenate([np.arange(67, 195), np.arange(3, 67), np.arange(3)])
    wqkT = np.concatenate([f91_w, f92_w], axis=0)[:, perm].T  # [195, 384]
    wb[:, _WB_QKA:_WB_QKA + 384] = wqkT[0:128]
    wb[0:67, _WB_QKB:_WB_QKB + 384] = wqkT[128:195]
    rh = _resize_mat(448, 56)  # [448, 56]; same matrix for H and W
    rhr = rh.reshape(4, 112, 56).transpose(1, 0, 2).reshape(112, 224)
    wb[0:112, _WB_RH:_WB_RH + 224] = rhr
    wb[0:112, _WB_RW:_WB_RW + 224] = rhr
    # fc8 bf16 hi/lo packed [128, 4ck, 8]: cols [hi(4) | lo(4)]
    fhi = fc8_w.astype(BFNP).astype(np.float32)
    flo = fc8_w - fhi
    fhiT = fhi.T.reshape(4, 128, 4).transpose(1, 0, 2)  # [p, ck, o]
    floT = flo.T.reshape(4, 128, 4).transpose(1, 0, 2)
    wb[:, _WB_FC8:_WB_FC8 + 32] = np.concatenate(
        [fhiT, floT], axis=2).reshape(128, 32)
    wb = wb.astype(BFNP)

    # x2 resize quadrants: pure gather (no arithmetic on the input) plus a
    # CONSTANT coefficient tensor; the multiply-adds happen on-device.
    y0, y1, wy = _resize_grid(112, 56)
    wx = wy
    x2b = x2.reshape(n, 128, 112, 112).astype(BFNP)
    quads = [x2b[:, :, y0, :][:, :, :, y0], x2b[:, :, y0, :][:, :, :, y1],
             x2b[:, :, y1, :][:, :, :, y0], x2b[:, :, y1, :][:, :, :, y1]]
    x2q = np.ascontiguousarray(
        np.stack(quads, axis=2).reshape(n, 128, 4 * HW))
    cw = [np.outer(1.0 - wy, 1.0 - wx), np.outer(1.0 - wy, wx),
          np.outer(wy, 1.0 - wx), np.outer(wy, wx)]
    cq = np.stack(cw).reshape(1, 4 * HW).astype(np.float32).astype(BFNP)
    cq = np.ascontiguousarray(np.broadcast_to(cq, (128, 4 * HW)))

    xb = np.ascontiguousarray(
        x.reshape(n, 3, 4, 112, 448).transpose(0, 3, 2, 1, 4)
        .reshape(n, 112, 4 * 3 * 448).astype(BFNP))
    d3b = np.ascontiguousarray(deep3.reshape(n, 320, HW).astype(BFNP))
    x4f = _4.reshape(n, 512, HW)
    x4h = x4f.astype(BFNP)
    x4l = (x4f - x4h.astype(np.float32)).astype(BFNP)

    shared = {"wb16": wb, "cq": cq}
    in_maps = []
    for i in range(n):
        m = dict(shared)
        m["x2q"] = x2q[i]
        m["x"] = xb[i]
        m["deep3"] = d3b[i]
        m["x4h"] = np.ascontiguousarray(x4h[i])
        m["x4l"] = np.ascontiguousarray(x4l[i])
        in_maps.append(m)
    return in_maps


def _install_ntff_hook() -> bool:
    """Register the NTFF profile hook that the agent image's antenv lacks."""
    try:
        import types

        import antenv

        if "antenv.axon_hooks" not in sys.modules:
            mod = types.ModuleType("antenv.axon_hooks")
            store = {"h": None}
            mod.set_axon_ntff_profile_hook = lambda h: store.update(h=h)
            mod.get_axon_ntff_profile_hook = lambda: store["h"]
            sys.modules["antenv.axon_hooks"] = mod
            antenv.axon_hooks = mod
            from trn_agent_boot.trn_boot import _ntff_profile_via_ctypes

            hook = _ntff_profile_via_ctypes("/opt/axon/libaxon_pjrt.so")
            if hook is None:
                return False
            mod.set_axon_ntff_profile_hook(hook)
        return sys.modules["antenv.axon_hooks"].get_axon_ntff_profile_hook() is not None
    except Exception as e:  # profiling is best-effort
        print(f"ntff hook install failed: {e}", file=sys.stderr)
        return False


def kernel(**inputs) -> np.ndarray:
    nc = _get_program()
    in_maps = _host_prep(inputs)
    trace = bool(int(os.environ.get("KERNEL_PROFILE", "0")))
    if trace:
        trace = _install_ntff_hook()
    res = run_bass_kernel_spmd(nc, in_maps, core_ids=list(range(N_CORES)),
                               trace=trace)
    _CACHE["last_result"] = res
    out = np.stack([r["out"] for r in res.results]).reshape(8, 4, 56, 56)
    return out.astype(np.float32)
